# revision 4
# baseline (speedup 1.0000x reference)
"""DiscriminativeLoss on 8 Trainium2 NeuronCores (Bass/Tile, SPMD) — v3.

Sharding: data-parallel over batch with pixel-split pairs — core k handles
sample k//2, half k%2 of the H*W pixels (X = 262144 px/core).

Layout (fp8e4 on the wire):
  a1 [64*128, 32*44]  pixel-major [m(8) | 1 | e(32) | pad(3)] per px, for
                      pass-1 masked sums via PE (4-way column tiling).
  e2 [128, 65536]     d-major e, rows (ph*32+d), SBUF-resident for pass 2.
  m2 [128, 16384]     masks in pass-2 tile layout rows (j*32+ph*8+c).

Pass 1: 2048 accumulating matmuls (lhsT=m, rhs=[1|e]) round-robin over 4 PE
column tiles; one f32 matmul combines the 4 partial [8,33] blocks; pairwise
AllReduce; mu/musq/w1/bias derived on device with tiny matmuls. While pass-1
runs (it is DMA/PE bound), DVE+ACT precompute esq = e*e (bf16 — exact for
fp8 inputs) for the first PRE pass-2 iters into a resident buffer.
Pass 2: T2=16 iters of 16384 px: 16 col-tiled matmuls -> ps[128,1024] =
e_sq - 2 e.mu (2 PSUM banks), ACT Sqrt(ps + musq) -> bf16, and one fused
custom DVE op sq(relu(s*m - delta)) with free-dim accumulation. esq for
iters >= PRE is computed inline, pipelined 3 ahead, split DVE/ACT.
Host only shards/casts inputs and does the O(C^2*D) finalization.
"""
from contextlib import ExitStack
from operator import add

import numpy as np

import concourse.bacc as bacc
import concourse.tile as tile
import concourse.bass as bass
from concourse import mybir
from concourse.bass_utils import run_bass_kernel_spmd

# ---- custom DVE op: out = relu(in0*in1 - s0)^2, accum_out = sum over free
import concourse.dve_ops as dve_ops_mod
from concourse.dve_ops import DveOp
from concourse.dve_spec import Spec, Src0, Src1, C0, Zero, relu, sq, lower, \
    _has_src1 as has_src1
from concourse.dve_uop import DveOpSpec
from concourse.dve_table_gen import dve_ver_for


def _dve_relu_np(x):
    return np.maximum(np.nan_to_num(x, nan=0.0, posinf=np.inf, neginf=-np.inf), 0)


def _varm_ref(in0, in1, s0, s1, imm2):
    b = (_dve_relu_np(in0.astype(np.float32) * in1.astype(np.float32) - s0) ** 2)
    return b, b.reshape(b.shape[0], -1).sum(axis=-1, keepdims=True)


def _register_varm_op():
    name = "VAR_MARGIN_ANT"
    for op in dve_ops_mod.OPS:
        if op.name == name:
            return op
    spec = Spec(body=sq(relu(Src0 * Src1 - C0)), accum=add, accum_init=Zero,
                reference=_varm_ref)
    ver = dve_ver_for("TRN2")
    opcode = max(dve_ops_mod._SUB_OPCODE_FOR_NAME.values()) + 1
    sha = DveOpSpec(name=name, opcode=opcode, uops=lower(spec, ver=ver),
                    rd1_en=has_src1(spec)).sha(ver)
    op = DveOp(name, spec, subdim=False, uops_sha={ver: sha})
    dve_ops_mod.OPS.append(op)
    dve_ops_mod._SUB_OPCODE_FOR_NAME[name] = opcode
    dve_ops_mod.CUSTOM_DVE_SPECS[name] = spec
    return op


# problem constants
B, D, H, W, C = 4, 32, 512, 1024, 8
HW = H * W
X = HW // 2               # pixels per core = 262144
NT = X // 128             # pass-1 pixel tiles = 2048
GJ = 32                   # pass-1 tiles per DMA group
G1 = NT // GJ             # pass-1 DMA groups = 64
P1C = 41                  # [m(8) | 1 | e(32)]
NQ = X // 4               # pixels per phase = 65536
T2 = 16                   # pass-2 iters
QI = NQ // T2             # q-cols per pass-2 iter = 4096
EW = QI // 4              # elementwise cols per iter = 1024
DELTA_VAR = 0.5
DELTA_DIST = 1.5
ALPHA, BETA, GAMMA = 1.0, 1.0, 0.001
EPS = 1e-12
N_CORES = 8

F32 = mybir.dt.float32
BF16 = mybir.dt.bfloat16
F8 = mybir.dt.float8e4
NP_F8 = mybir.dt.np(F8)


def build_module(reps: int = 1, use_loop: bool | None = None,
                 skip_ar: bool = False, do_pass1: bool = True,
                 do_pass2: bool = True, pre: int = 8, ahead: int = 3,
                 esq_dve_cols: int = 2048, p1c: int = P1C, p1bufs: int = 4):
    """Build + compile the SPMD Bass module. reps>1 wraps the two heavy
    loops in hardware For_i (identical work per iteration) for timing."""
    varm = _register_varm_op()
    nc = bacc.Bacc("TRN2", target_bir_lowering=False, debug=False,
                   num_devices=N_CORES)

    a1 = nc.dram_tensor("a1", [G1 * 128, GJ * p1c], F8, kind="ExternalInput")
    e2 = nc.dram_tensor("e2", [128, NQ], F8, kind="ExternalInput")
    m2 = nc.dram_tensor("m2", [128, T2 * EW], F8, kind="ExternalInput")
    var_out = nc.dram_tensor("var_out", [128, 1], F32, kind="ExternalOutput")
    mu_out = nc.dram_tensor("mu_out", [8, 32], F32, kind="ExternalOutput")

    # inline constants
    w2_np = np.kron(np.eye(4, dtype=np.float32), np.ones((32, 8), np.float32))
    w2_dram = nc.inline_tensor(
        np.ascontiguousarray(w2_np.astype(mybir.dt.np(BF16))), "w2ones")
    eyem2_dram = nc.inline_tensor(-2.0 * np.eye(8, dtype=np.float32), "eyem2")
    sel128_np = np.zeros((128, 8), np.float32)
    for p in range(128):
        if p % 32 < 8:
            sel128_np[p, p % 32] = 1.0
    sel128_dram = nc.inline_tensor(sel128_np, "sel128")
    sel32_dram = nc.inline_tensor(
        np.ascontiguousarray(np.tile(np.eye(8, dtype=np.float32), (1, 4))), "sel32")

    with tile.TileContext(nc) as tc, ExitStack() as ctx:
        p1pool = ctx.enter_context(tc.tile_pool(name="p1", bufs=p1bufs))
        ps1 = ctx.enter_context(tc.tile_pool(name="ps1", bufs=1, space="PSUM"))
        psS = ctx.enter_context(tc.tile_pool(name="psS", bufs=1, space="PSUM"))
        small = ctx.enter_context(tc.tile_pool(name="small", bufs=1))
        dram = ctx.enter_context(tc.tile_pool(name="dram", bufs=1, space="DRAM"))
        res = ctx.enter_context(tc.tile_pool(name="res", bufs=1))
        esqpool = ctx.enter_context(tc.tile_pool(name="esq", bufs=ahead + 1))
        spool = ctx.enter_context(tc.tile_pool(name="sp", bufs=3))
        ps2pool = ctx.enter_context(tc.tile_pool(name="ps2", bufs=3, space="PSUM"))

        # constants into SBUF (tiny DMAs, issued first)
        w2sb = res.tile([128, 32], BF16)
        nc.sync.dma_start(w2sb[:], w2_dram[:])
        selsb = res.tile([128, 8], F32)
        nc.sync.dma_start(selsb[:], sel128_dram[:])
        sel32sb = res.tile([8, 32], F32)
        nc.sync.dma_start(sel32sb[:], sel32_dram[:])
        eyem2sb = res.tile([8, 8], F32)
        nc.sync.dma_start(eyem2sb[:], eyem2_dram[:])

        # resident pass-2 inputs, loaded in big chunks interleaved with pass 1
        e2sb = res.tile([128, NQ], F8)
        m2sb = res.tile([128, T2 * EW], F8)
        # e2 first half + m2 first half load during pass-1 (feed esq
        # precompute + pass-2 start); the rest loads inside the pass-2 loop
        # where the DMA engines are otherwise idle.
        chunks = [("e", 0), ("e", 1), ("e", 2), ("e", 3)]
        p2chunks = [("m", 0), ("e", 4), ("e", 5), ("e", 6), ("e", 7), ("m", 1)]

        def issue_chunk(ch):
            kind, i = ch
            if kind == "e":
                cw = NQ // 8
                nc.sync.dma_start(e2sb[:, i * cw:(i + 1) * cw],
                                  e2[:, i * cw:(i + 1) * cw])
            else:
                cw = T2 * EW // 2
                nc.sync.dma_start(m2sb[:, i * cw:(i + 1) * cw],
                                  m2[:, i * cw:(i + 1) * cw])

        # esq precompute target for pass-2 iters [0, pre)
        if pre > 0:
            esq_pre = res.tile([128, pre * QI], BF16, tag="esq_pre")
        else:
            esq_pre = None

        def make_esq_into(dst, t):
            """dst[:, 0:QI] = e2sb[:, t*QI:(t+1)*QI]^2, split DVE/ACT."""
            dv = esq_dve_cols
            src_d = e2sb[:, t * QI:t * QI + dv]
            nc.vector.tensor_mul(dst[:, 0:dv], src_d, src_d)
            nc.scalar.activation(dst[:, dv:QI], e2sb[:, t * QI + dv:(t + 1) * QI],
                                 mybir.ActivationFunctionType.Square)

        num_ps = ps1.tile([128, 33], F32)
        chunk_at = {4: 0, 20: 1, 36: 2, 52: 3}

        # ---- pass 1 (+ esq precompute riding the idle DVE/ACT) ----
        def pass1_body(_iv=None):
            nxt = [0]
            for g in range(G1):
                big = p1pool.tile([128, GJ * p1c], F8)
                nc.sync.dma_start(big[:], a1[g * 128:(g + 1) * 128, :])
                if g in chunk_at and nxt[0] < len(chunks):
                    issue_chunk(chunks[nxt[0]])
                    nxt[0] += 1
                for j in range(GJ):
                    t = g * GJ + j
                    tp = t % 4
                    nc.tensor.matmul(
                        num_ps[32 * tp:32 * tp + 8, :],
                        lhsT=big[:, j * p1c:j * p1c + 8],
                        rhs=big[:, j * p1c + 8:j * p1c + 41],
                        start=(t < 4), stop=(t >= NT - 4),
                        tile_position=(0, 32 * tp),
                    )
            while nxt[0] < len(chunks):
                issue_chunk(chunks[nxt[0]])
                nxt[0] += 1
            if esq_pre is not None:
                for t in range(min(pre, T2)):
                    make_esq_into(esq_pre[:, t * QI:(t + 1) * QI], t)

        loop = (reps > 1) if use_loop is None else use_loop
        if do_pass1:
            if loop:
                with tc.For_i(0, reps, 1) as _i:
                    pass1_body()
            else:
                pass1_body()

            # combine the 4 column-tile partials: [128,33] -> [8,33]
            num_sb = small.tile([128, 33], F32)
            nc.vector.tensor_copy(num_sb[:], num_ps[:])
            comb_ps = psS.tile([8, 33], F32, tag="prep")
            nc.tensor.matmul(comb_ps[:], lhsT=selsb[:], rhs=num_sb[:],
                             start=True, stop=True, tile_position=(0, 0))
            red = small.tile([8, 33], F32)
            if skip_ar:
                nc.scalar.mul(red[:], comb_ps[:], 2.0)
            else:
                comb_sb = small.tile([8, 33], F32)
                nc.vector.tensor_copy(comb_sb[:], comb_ps[:])
                cc_in = dram.tile([8, 33], F32)
                cc_out = dram.tile([8, 33], F32)
                nc.sync.dma_start(cc_in[:], comb_sb[:])
                nc.gpsimd.collective_compute(
                    "AllReduce", mybir.AluOpType.add,
                    replica_groups=[[0, 1], [2, 3], [4, 5], [6, 7]],
                    ins=[cc_in.opt()], outs=[cc_out.opt()],
                )
                nc.sync.dma_start(red[:], cc_out[:])

            # derive mu, musq
            recip = small.tile([8, 1], F32)
            nc.vector.reciprocal(recip[:], red[:, 0:1])
            mu = small.tile([8, 32], F32)
            nc.vector.tensor_scalar_mul(mu[:], red[:, 1:33], recip[:])
            nc.sync.dma_start(mu_out.ap(), mu[:])
            musq_scr = small.tile([8, 32], F32)
            musq = small.tile([8, 1], F32)
            nc.scalar.activation(musq_scr[:], mu[:],
                                 mybir.ActivationFunctionType.Square,
                                 accum_out=musq[:])
        else:
            for ch in chunks:
                issue_chunk(ch)
            if esq_pre is not None:
                for t in range(min(pre, T2)):
                    make_esq_into(esq_pre[:, t * QI:(t + 1) * QI], t)
            mu = small.tile([8, 32], F32)
            nc.vector.memset(mu[:], 0.01)
            musq = small.tile([8, 1], F32)
            nc.vector.memset(musq[:], 0.0032)

        # w1 = block-diag(-2 mu^T) [128, 32] fp8, assembled via 4 tiny matmuls
        w1ps = psS.tile([128, 32], F32, tag="prep")
        for ph in range(4):
            nc.tensor.matmul(w1ps[32 * ph:32 * ph + 32, 8 * ph:8 * ph + 8],
                             lhsT=mu[:], rhs=eyem2sb[:],
                             start=True, stop=True, tile_position=(0, 32 * ph))
        w1sb = res.tile([128, 32], F8)
        nc.vector.memset(w1sb[:], 0.0)
        for ph in range(4):
            nc.vector.tensor_copy(w1sb[32 * ph:32 * ph + 32, 8 * ph:8 * ph + 8],
                                  w1ps[32 * ph:32 * ph + 32, 8 * ph:8 * ph + 8])

        # biasq[p] = musq[p%8]  [128,1] f32 via 4 tiny matmuls
        bps = psS.tile([128, 1], F32, tag="prep")
        for jj in range(4):
            nc.tensor.matmul(bps[32 * jj:32 * jj + 32, :], lhsT=sel32sb[:],
                             rhs=musq[:], start=True, stop=True,
                             tile_position=(0, 32 * jj))
        biasq = res.tile([128, 1], F32)
        nc.vector.tensor_copy(biasq[:], bps[:])

        # ---- pass 2 ----
        if do_pass2:
            partials = res.tile([128, T2], F32)
            scr = res.tile([128, EW], BF16)

            def pass2_body(_iv=None):
                for ch in p2chunks:
                    issue_chunk(ch)
                inflight = {}
                for tn in range(pre, min(pre + ahead, T2)):
                    dst = esqpool.tile([128, QI], BF16)
                    make_esq_into(dst, tn)
                    inflight[tn] = dst
                for t in range(T2):
                    tn = t + ahead
                    if t >= pre and tn < T2:
                        dst = esqpool.tile([128, QI], BF16)
                        make_esq_into(dst, tn)
                        inflight[tn] = dst
                    if t < pre:
                        esq = esq_pre[:, t * QI:(t + 1) * QI]
                    else:
                        esq = inflight.pop(t)[:]
                    ps = ps2pool.tile([128, EW], F32)
                    for j in range(4):
                        for h2 in range(2):
                            lo = j * EW + h2 * 512
                            nc.tensor.matmul(
                                ps[32 * j:32 * j + 32, h2 * 512:h2 * 512 + 512],
                                lhsT=w2sb[:], rhs=esq[:, lo:lo + 512],
                                start=True, stop=False, tile_position=(0, 32 * j))
                            nc.tensor.matmul(
                                ps[32 * j:32 * j + 32, h2 * 512:h2 * 512 + 512],
                                lhsT=w1sb[:],
                                rhs=e2sb[:, t * QI + lo:t * QI + lo + 512],
                                start=False, stop=True, tile_position=(0, 32 * j))
                    s = spool.tile([128, EW], BF16)
                    nc.scalar.activation(s[:], ps[:],
                                         mybir.ActivationFunctionType.Sqrt,
                                         bias=biasq[:])
                    nc.vector._custom_dve(
                        varm, out=scr[:], in0=s[:],
                        in1=m2sb[:, t * EW:(t + 1) * EW],
                        s0=float(DELTA_VAR),
                        accum_out=partials[:, t:t + 1])

            if loop:
                with tc.For_i(0, reps, 1) as _i:
                    pass2_body()
            else:
                pass2_body()

            var_sb = small.tile([128, 1], F32, tag="var_sb")
            nc.vector.reduce_sum(var_sb[:], partials[:],
                                 axis=mybir.AxisListType.X)
            nc.sync.dma_start(var_out.ap(), var_sb[:])

    nc.compile()
    return nc


def host_prep(embeddings: np.ndarray, instance_masks: np.ndarray,
              p1c: int = P1C):
    """Shard + lay out inputs for the 8 cores (layout/cast only)."""
    e_all = np.asarray(embeddings, dtype=np.float32).reshape(B, D, HW)
    m_all = np.asarray(instance_masks).reshape(B, C, HW).astype(np.float32)
    in_maps = []
    for k in range(N_CORES):
        b, h = k // 2, k % 2
        e_h = e_all[b, :, h * X:(h + 1) * X]        # [32, X]
        m_h = m_all[b, :, h * X:(h + 1) * X]        # [8, X]
        p1 = np.zeros((X, p1c), dtype=np.float32)
        p1[:, 0:8] = m_h.T
        p1[:, 8] = 1.0
        p1[:, 9:41] = e_h.T
        a1 = (p1.reshape(G1, GJ, 128, p1c)
                .transpose(0, 2, 1, 3)
                .reshape(G1 * 128, GJ * p1c)
                .astype(NP_F8))
        e2 = np.ascontiguousarray(
            e_h.reshape(D, 4, NQ).transpose(1, 0, 2).reshape(128, NQ)
            .astype(NP_F8))
        # m2[j*32+ph*8+c, t*EW+r] = m[c, ph*NQ + t*QI + j*EW + r]
        m2 = np.ascontiguousarray(
            m_h.reshape(C, 4, T2, 4, EW).transpose(3, 1, 0, 2, 4)
               .reshape(128, T2 * EW).astype(NP_F8))
        in_maps.append({"a1": a1, "e2": e2, "m2": m2})
    return in_maps


def host_finalize(results):
    """Combine per-core outputs into the scalar loss (float64 internally)."""
    per_sample = np.empty(B, dtype=np.float64)
    n_pairs = C * (C - 1) / 2.0
    for b in range(B):
        v = (results[2 * b]["var_out"].astype(np.float64).reshape(16, 8)
             + results[2 * b + 1]["var_out"].astype(np.float64).reshape(16, 8))
        var_per_cluster = v.sum(axis=0) / HW          # [C]
        var_loss = var_per_cluster.sum() / C
        mu = results[2 * b]["mu_out"].astype(np.float64)   # [C, D]
        diff = mu[:, None, :] - mu[None, :, :]
        dist = np.sqrt((diff * diff).sum(-1) + EPS)
        pair = np.maximum(DELTA_DIST - dist, 0.0) ** 2
        iu = np.triu_indices(C, k=1)
        dist_loss = pair[iu].sum() / n_pairs
        reg_loss = np.mean(np.sqrt((mu * mu).sum(-1) + EPS))
        per_sample[b] = ALPHA * var_loss + BETA * dist_loss + GAMMA * reg_loss
    return np.float32(per_sample.mean())


_CACHE = {}


def kernel(embeddings: np.ndarray, instance_masks: np.ndarray) -> np.ndarray:
    if "nc" not in _CACHE:
        _CACHE["nc"] = build_module(reps=1)
    nc = _CACHE["nc"]
    in_maps = host_prep(embeddings, instance_masks)
    res = run_bass_kernel_spmd(nc, in_maps, list(range(N_CORES)))
    return host_finalize(res.results)
